# revision 1
# baseline (speedup 1.0000x reference)
"""Decode-step multi-head attention with KV cache (DeepSpeed-inference style).

Full shapes (hardcoded per problem spec):
  query/key/value: [16, 1, 2048] f32
  key_cache/value_cache: [16, 16, 4096, 128] f32
  cache_len: scalar int (2048)
Output: [16, 1, 2048] f32

Strategy: data-parallel over batch across 8 NeuronCores (2 batches/core =
32 (batch, head) pairs per core). Per pair, the core streams the K and V
cache slices ([cache_len, 128] each) from HBM, computes scores with fused
multiply+reduce on VectorE (K stays in its natural [k, d] layout), exp via
ScalarE (with fused row-sum for the softmax denominator), and aggregates
V with TensorE matmuls (contraction over the k partition axis). The new
token's score/value is folded in as an extra column / extra matmul. The
softmax denominator is reduced across partitions with a ones-vector
matmul; division happens once at the end in pair-major layout.
"""

import functools
import os
from contextlib import ExitStack

import numpy as np

import concourse.bacc as bacc
import concourse.bass as bass
import concourse.mybir as mybir
import concourse.tile as tile
from concourse import bass_utils

N_CORES = 8
P = 128  # partitions
NEG_BIG = -1e30

# test.py hooks: set TRACE=True before calling kernel() to collect a profile.
TRACE = False
TRACE_KWARGS = {}
LAST_RESULTS = None


def _build_program(bl: int, n_heads: int, max_seq: int, hd: int, cache_len: int):
    """Build + compile the per-core program. bl = local batch count."""
    npairs = bl * n_heads
    assert npairs <= 32  # epilogue (32x32 DVE transpose for l) assumes this
    assert hd == P
    nch = cache_len // P          # full 128-row chunks of the cache
    rem = cache_len - nch * P     # remainder rows
    ncht = nch + (1 if rem else 0)
    sm_scale = 1.0 / float(np.sqrt(hd))

    nc = bacc.Bacc("TRN2", target_bir_lowering=False, debug=False)
    f32 = mybir.dt.float32
    f16 = mybir.dt.float16

    kc = nc.dram_tensor("kc", [bl, n_heads, max_seq, hd], f32, kind="ExternalInput").ap()
    vc = nc.dram_tensor("vc", [bl, n_heads, max_seq, hd], f32, kind="ExternalInput").ap()
    q = nc.dram_tensor("q", [npairs, hd], f32, kind="ExternalInput").ap()
    kn = nc.dram_tensor("kn", [npairs, hd], f32, kind="ExternalInput").ap()
    vn = nc.dram_tensor("vn", [npairs, hd], f32, kind="ExternalInput").ap()
    ident = nc.dram_tensor("ident", [P, P], f32, kind="ExternalInput").ap()
    out = nc.dram_tensor("out", [npairs, hd], f32, kind="ExternalOutput").ap()

    with tile.TileContext(nc) as tc, ExitStack() as ctx:
        singles = ctx.enter_context(tc.tile_pool(name="singles", bufs=1))
        kbufs = int(os.environ.get("KBUFS", "6"))
        kpool = ctx.enter_context(tc.tile_pool(name="kpool", bufs=kbufs))
        vpool = ctx.enter_context(tc.tile_pool(name="vpool", bufs=kbufs))
        ppool = ctx.enter_context(tc.tile_pool(name="ppool", bufs=3))
        stats = ctx.enter_context(tc.tile_pool(name="stats", bufs=6))
        psum_o = ctx.enter_context(tc.tile_pool(name="psum_o", bufs=5, space="PSUM"))
        psum_1 = ctx.enter_context(tc.tile_pool(name="psum_1", bufs=1, space="PSUM"))

        def emit_loads(b, h):
            kt = kpool.tile([P, ncht, hd], f32, tag="kt")
            # V is cast to fp16 during the DMA (SWDGE): halves PE matmul
            # passes; psum accumulation stays fp32.
            vt = vpool.tile([P, ncht, hd], f16, tag="vt")
            if nch:
                kslc = kc[b, h, 0 : nch * P, :].rearrange("(p c) d -> p c d", c=nch)
                vslc = vc[b, h, 0 : nch * P, :].rearrange("(p c) d -> p c d", c=nch)
                nc.sync.dma_start(out=kt[:, :nch, :], in_=kslc)
                nc.gpsimd.dma_start(out=vt[:, :nch, :], in_=vslc)
            if rem:
                # zero V pad first so (p == 0) x garbage cannot produce NaN
                nc.gpsimd.memset(vt[:, nch, :], 0.0)
                nc.sync.dma_start(out=kt[:rem, nch, :], in_=kc[b, h, nch * P : cache_len, :])
                nc.gpsimd.dma_start(out=vt[:rem, nch, :], in_=vc[b, h, nch * P : cache_len, :])
            return kt, vt

        # issue the first pairs' streaming loads before any setup traffic
        PRELOAD = 0
        preloaded = [emit_loads(*divmod(p, n_heads)) for p in range(min(PRELOAD, npairs))]

        ones_col = singles.tile([P, 1], f32)
        nc.vector.memset(ones_col, 1.0)

        ident_sb = singles.tile([P, P], f32)
        nc.sync.dma_start(out=ident_sb, in_=ident)

        kn_all = singles.tile([npairs, hd], f32)
        nc.sync.dma_start(out=kn_all, in_=kn)
        vn_all = singles.tile([npairs, hd], f32)
        nc.sync.dma_start(out=vn_all, in_=vn)
        q_all = singles.tile([npairs, hd], f32)
        nc.sync.dma_start(out=q_all, in_=q)

        # all queries broadcast to every partition, once:
        # q_all_b[j, p, d] = q[p, d]
        q_all_b = singles.tile([P, npairs, hd], f32)
        q_bsrc = bass.AP(tensor=q.tensor, offset=q.offset, ap=[[0, P]] + q.ap)
        nc.gpsimd.dma_start(out=q_all_b, in_=q_bsrc)

        # Softmax denominators, one column per pair (partition 0).
        lrow = psum_1.tile([1, npairs], f32)
        # Unnormalized cache-part outputs, head-dim on partitions, one
        # column per pair.
        out_sb = singles.tile([P, npairs], f32)

        # ---- new-token contribution, batched over all pairs ----
        prod_new = singles.tile([npairs, hd], f32)
        nc.vector.tensor_mul(prod_new, kn_all, q_all)
        s_new = singles.tile([npairs, 1], f32)
        nc.vector.reduce_sum(s_new, prod_new, axis=mybir.AxisListType.X)
        p_new = singles.tile([npairs, 1], f32)
        nc.scalar.activation(
            out=p_new, in_=s_new, func=mybir.ActivationFunctionType.Exp, scale=sm_scale
        )
        # rows 0..npairs-1: p_new[p] * v_new[p]; rest zero
        vns = singles.tile([P, hd], f32)
        nc.vector.memset(vns, 0.0)
        nc.vector.tensor_scalar_mul(vns[:npairs, :], vn_all, p_new)
        vnsT_ps = psum_1.tile([P, P], f32)
        nc.tensor.transpose(vnsT_ps, vns, ident_sb)
        vnsT = singles.tile([P, npairs], f32)
        nc.scalar.copy(vnsT, vnsT_ps[:, :npairs])

        for p in range(npairs):
            b, h = divmod(p, n_heads)

            kt, vt = preloaded[p] if p < len(preloaded) else emit_loads(b, h)

            q_b = q_all_b[:, p, :]

            # scores: s[kpart, c] = sum_d K[k, d] * q[d]
            s_tile = stats.tile([P, ncht], f32, tag="s")
            prod = ppool.tile([P, ncht, hd], f32, tag="prod")
            if rem:
                nc.gpsimd.memset(s_tile[:, nch : nch + 1], NEG_BIG)
            if nch:
                q_bb = bass.AP(
                    tensor=q_b.tensor,
                    offset=q_b.offset,
                    ap=[q_b.ap[0], [0, nch], q_b.ap[1]],
                )
                nc.vector.tensor_mul(prod[:, :nch, :], kt[:, :nch, :], q_bb)
                nc.vector.reduce_sum(
                    s_tile[:, :nch], prod[:, :nch, :], axis=mybir.AxisListType.X
                )
            if rem:
                nc.vector.tensor_mul(prod[:rem, nch, :], kt[:rem, nch, :], q_b[:rem, :])
                nc.vector.reduce_sum(
                    s_tile[:rem, nch : nch + 1],
                    prod[:rem, nch, :],
                    axis=mybir.AxisListType.X,
                )

            # softmax numerator (scores scaled here) + fused per-partition sums
            p_tile = stats.tile([P, ncht], f16, tag="p")
            l_part = stats.tile([P, 1], f32, tag="l")
            nc.scalar.activation(
                out=p_tile,
                in_=s_tile,
                func=mybir.ActivationFunctionType.Exp,
                scale=sm_scale,
                accum_out=l_part,
            )

            # V aggregation: out[d] = sum_k p[k] V[k, d]
            acc = psum_o.tile([P, 1], f32, tag="acc")
            for c in range(ncht):
                nc.tensor.matmul(
                    acc,
                    lhsT=vt[:, c, :],
                    rhs=p_tile[:, c : c + 1],
                    start=(c == 0),
                    stop=(c == ncht - 1),
                )

            # softmax denominator (cache part): sum l_part over partitions
            nc.tensor.matmul(
                lrow[0:1, p : p + 1], lhsT=ones_col, rhs=l_part, start=True, stop=True
            )

            nc.scalar.copy(out_sb[:, p : p + 1], acc)

        # ---- epilogue: add new-token contribution, normalize, emit ----
        out_full = singles.tile([P, npairs], f32)
        nc.vector.tensor_add(out_full, out_sb, vnsT)

        l32 = singles.tile([32, 32], f32)
        nc.vector.memset(l32, 0.0)
        nc.scalar.copy(l32[0:1, :npairs], lrow)
        l32t = singles.tile([32, 32], f32)
        nc.vector.transpose(l32t, l32)
        l_tot = singles.tile([npairs, 1], f32)
        nc.vector.tensor_add(l_tot, l32t[:npairs, 0:1], p_new)
        recip_l = singles.tile([npairs, 1], f32)
        nc.vector.reciprocal(recip_l, l_tot)

        oT = psum_1.tile([npairs, hd], f32)
        nc.tensor.transpose(oT, out_full, ident_sb)

        final_sb = singles.tile([npairs, hd], f32)
        nc.scalar.mul(final_sb, oT, mul=recip_l)
        nc.sync.dma_start(out=out, in_=final_sb)

    nc.compile()
    return nc


@functools.lru_cache(maxsize=4)
def _program(bl, n_heads, max_seq, hd, cache_len):
    return _build_program(bl, n_heads, max_seq, hd, cache_len)


def kernel(query, key, value, key_cache, value_cache, cache_len):
    global LAST_RESULTS
    query = np.asarray(query, dtype=np.float32)
    key = np.asarray(key, dtype=np.float32)
    value = np.asarray(value, dtype=np.float32)
    key_cache = np.asarray(key_cache, dtype=np.float32)
    value_cache = np.asarray(value_cache, dtype=np.float32)
    cache_len = int(cache_len)

    b_sz, q_len, d_model = query.shape
    _, n_heads, max_seq, hd = key_cache.shape
    assert q_len == 1 and d_model == n_heads * hd
    assert b_sz % N_CORES == 0
    bl = b_sz // N_CORES

    prog = _program(bl, n_heads, max_seq, hd, cache_len)

    ident = np.eye(P, dtype=np.float32)
    in_maps = []
    for i in range(N_CORES):
        sl = slice(i * bl, (i + 1) * bl)
        in_maps.append(
            {
                "kc": np.ascontiguousarray(key_cache[sl]),
                "vc": np.ascontiguousarray(value_cache[sl]),
                "q": np.ascontiguousarray(query[sl]).reshape(bl * n_heads, hd),
                "kn": np.ascontiguousarray(key[sl]).reshape(bl * n_heads, hd),
                "vn": np.ascontiguousarray(value[sl]).reshape(bl * n_heads, hd),
                "ident": ident,
            }
        )

    try:
        res = bass_utils.run_bass_kernel_spmd(
            prog, in_maps, core_ids=list(range(N_CORES)), trace=TRACE, **TRACE_KWARGS
        )
    except Exception:
        # A previously crashed NeuronCore can leave the first execution
        # attempt failing with a transient runtime error; retry once.
        res = bass_utils.run_bass_kernel_spmd(
            prog, in_maps, core_ids=list(range(N_CORES)), trace=TRACE, **TRACE_KWARGS
        )
    LAST_RESULTS = res
    outs = [res.results[i]["out"].reshape(bl, q_len, d_model) for i in range(N_CORES)]
    return np.concatenate(outs, axis=0)



# revision 2
# speedup vs baseline: 1.0487x; 1.0487x over previous
"""Decode-step multi-head attention with KV cache (DeepSpeed-inference style).

Full shapes (hardcoded per problem spec):
  query/key/value: [16, 1, 2048] f32
  key_cache/value_cache: [16, 16, 4096, 128] f32
  cache_len: 2048
Output: [16, 1, 2048] f32

Data-parallel over batch across 8 NeuronCores: 2 batches/core = 32
(batch, head) pairs per core. Per pair the core streams the K cache
slice (f32, sync HWDGE queue) and the V cache slice (cast f32->f16
during the SWDGE DMA on the gpsimd queue) from HBM; the ~370 GB/s
per-core HBM read rate is the roofline (~64 MiB/core). All 63 streaming
DMAs are emitted first so both queues drain from t=0, with deep buffer
pools so neither queue stalls on consumption. Scores are computed on
DVE (mul + per-chunk reduce), exp with fused row-sum on ScalarE, and V
aggregation on TensorE as f16 matmuls accumulating in PSUM. The
new-token (q.k_new, v_new) contributions are folded in with two
batched diag(p_new) matmuls in the prologue. Softmax denominators
collect into a persistent [1, 64] PSUM row via ones-matmuls; the final
normalize broadcasts the reciprocal row across partitions with a
ones-matmul and multiplies on DVE. The output leaves as [hd, npairs]
and the host transposes. The last pair is split into 4 chunk-groups so
the post-stream compute tail stays short.
"""

import functools
import os
from contextlib import ExitStack

import numpy as np

import concourse.bacc as bacc
import concourse.bass as bass
import concourse.mybir as mybir
import concourse.tile as tile
from concourse import bass_utils

N_CORES = 8
P = 128

TRACE = False
TRACE_KWARGS = {}
LAST_RESULTS = None


def _build_program(bl: int, n_heads: int, max_seq: int, hd: int, cache_len: int):
    npairs = bl * n_heads
    assert hd == P and cache_len % P == 0 and npairs == 32
    nch = cache_len // P  # 16
    NSPLIT = int(os.environ.get("NSPLIT", "4"))  # chunk-groups for last pair
    assert nch % NSPLIT == 0
    spc = nch // NSPLIT  # chunks per split piece
    sm_scale = 1.0 / float(np.sqrt(hd))
    KBUFS = int(os.environ.get("KBUFS", "11"))
    VBUFS = int(os.environ.get("VBUFS", "8"))
    MSPLIT = int(os.environ.get("MSPLIT", "0"))

    nc = bacc.Bacc("TRN2", target_bir_lowering=False, debug=False)
    f32 = mybir.dt.float32
    f16 = mybir.dt.float16

    kc = nc.dram_tensor("kc", [bl, n_heads, max_seq, hd], f32, kind="ExternalInput").ap()
    vc = nc.dram_tensor("vc", [bl, n_heads, max_seq, hd], f32, kind="ExternalInput").ap()
    q = nc.dram_tensor("q", [npairs, hd], f32, kind="ExternalInput").ap()
    kn = nc.dram_tensor("kn", [npairs, hd], f32, kind="ExternalInput").ap()
    vn = nc.dram_tensor("vn", [npairs, hd], f32, kind="ExternalInput").ap()
    ident32 = nc.dram_tensor("ident32", [npairs, npairs], f32, kind="ExternalInput").ap()
    out = nc.dram_tensor("out", [hd, npairs], f32, kind="ExternalOutput").ap()

    with tile.TileContext(nc) as tc, ExitStack() as ctx:
        singles = ctx.enter_context(tc.tile_pool(name="singles", bufs=1))
        kpool = ctx.enter_context(tc.tile_pool(name="kpool", bufs=KBUFS))
        vpool = ctx.enter_context(tc.tile_pool(name="vpool", bufs=VBUFS))
        ppool = ctx.enter_context(tc.tile_pool(name="ppool", bufs=4))
        stats = ctx.enter_context(tc.tile_pool(name="stats", bufs=10))
        psum_acc = ctx.enter_context(tc.tile_pool(name="psum_acc", bufs=3, space="PSUM"))
        psum_new = ctx.enter_context(tc.tile_pool(name="psum_new", bufs=1, space="PSUM"))
        psum_q = ctx.enter_context(tc.tile_pool(name="psum_q", bufs=2, space="PSUM"))
        psum_l = ctx.enter_context(tc.tile_pool(name="psum_l", bufs=1, space="PSUM"))

        # ---- streaming loads: K f32 on sync HWDGE, V f32->f16 cast on
        # gpsimd SWDGE (16-bit V keeps the PE matmul weight loads fast).
        # Emitted first so both queues start draining at t=0; buffer reuse
        # (KBUFS/VBUFS) self-regulates queue depth. ----
        def emit_k(b, h, lo_ch, n_ch, tag, bufs):
            t = kpool.tile([P, n_ch, hd], f32, tag=tag, bufs=bufs, name=f"kt_{b}_{h}_{lo_ch}")
            src = kc[b, h, 0 : nch * P, :].rearrange("(p c) d -> p c d", c=nch)
            nc.sync.dma_start(out=t, in_=src[:, lo_ch : lo_ch + n_ch, :])
            return t

        def emit_v(b, h, lo_ch, n_ch, tag, bufs):
            t = vpool.tile([P, n_ch, hd], f16, tag=tag, bufs=bufs, name=f"vt_{b}_{h}_{lo_ch}")
            src = vc[b, h, 0 : nch * P, :].rearrange("(p c) d -> p c d", c=nch)
            nc.gpsimd.dma_start(out=t, in_=src[:, lo_ch : lo_ch + n_ch, :])
            return t

        kts, vts = [], []
        for p in range(npairs - 1):
            b, h = divmod(p, n_heads)
            kts.append(emit_k(b, h, 0, nch, "kt", KBUFS))
            vts.append(emit_v(b, h, 0, nch, "vt", VBUFS))
        b31, h31 = divmod(npairs - 1, n_heads)
        kt31 = [emit_k(b31, h31, j * spc, spc, "kt31", NSPLIT) for j in range(NSPLIT)]
        vt31 = [emit_v(b31, h31, j * spc, spc, "vt31", NSPLIT) for j in range(NSPLIT)]

        # ---- setup: small loads on the scalar HWDGE queue ----
        q_flat = singles.tile([1, npairs * hd], f32, tag="q_flat")
        q_flat_src = bass.AP(tensor=q.tensor, offset=q.offset, ap=[[0, 1], [1, npairs * hd]])
        nc.scalar.dma_start(out=q_flat, in_=q_flat_src)
        q_all = singles.tile([npairs, hd], f32, tag="q_all")
        nc.scalar.dma_start(out=q_all, in_=q)
        kn_all = singles.tile([npairs, hd], f32, tag="kn_all")
        nc.scalar.dma_start(out=kn_all, in_=kn)
        vn_all = singles.tile([npairs, hd], f32, tag="vn_all")
        nc.scalar.dma_start(out=vn_all, in_=vn)
        id32 = singles.tile([npairs, npairs], f32, tag="id32")
        nc.scalar.dma_start(out=id32, in_=ident32)

        ones_row = singles.tile([1, P], f32, tag="ones_row")
        nc.vector.memset(ones_row, 1.0)
        ones_col = singles.tile([P, 1], f32, tag="ones_col")
        nc.vector.memset(ones_col, 1.0)

        # ---- q broadcast to all partitions via PE ones-matmul:
        # qb[j][k, d'] = q_flat[0, j*512 + d'] for every partition k ----
        QCOLS = 512
        nqb = (npairs * hd) // QCOLS  # 8 blocks of 4 pairs each
        pairs_per_qb = QCOLS // hd  # 4
        qbs = []
        for j in range(nqb):
            psq = psum_q.tile([P, QCOLS], f32, tag="psq", name=f"psq{j}")
            nc.tensor.matmul(
                psq, lhsT=ones_row, rhs=q_flat[0:1, j * QCOLS : (j + 1) * QCOLS],
                start=True, stop=True,
            )
            qb = singles.tile([P, QCOLS], f32, tag=f"qb{j}", name=f"qb{j}")
            nc.scalar.copy(qb, psq)
            qbs.append(qb)

        # ---- new-token scores: p_new[p] = exp(q[p]·kn[p] * scale) ----
        prod_new = singles.tile([npairs, hd], f32, tag="prod_new")
        nc.vector.tensor_mul(prod_new, kn_all, q_all)
        s_new = singles.tile([npairs, 1], f32, tag="s_new")
        nc.vector.reduce_sum(s_new, prod_new, axis=mybir.AxisListType.X)
        p_new = singles.tile([npairs, 1], f32, tag="p_new")
        nc.scalar.activation(
            out=p_new, in_=s_new, func=mybir.ActivationFunctionType.Exp, scale=sm_scale
        )

        # Softmax denominators, partition 0: cols 0..31 = cache part (one
        # per pair), cols 32..63 = transposed p_new row. Persistent.
        lrow = psum_l.tile([1, 2 * npairs], f32, tag="lrow")
        # Unnormalized outputs, head-dim on partitions, one column per pair.
        out_sb = singles.tile([P, npairs], f32, tag="out_sb")

        # ---- new-token contributions, batched via diag(p_new) matmuls ----
        pd = singles.tile([npairs, npairs], f32, tag="pd")
        nc.vector.tensor_scalar_mul(pd, id32, p_new)
        # ps_new[d, p] = p_new[p] * vn[p, d]
        ps_new = psum_new.tile([P, npairs], f32, tag="ps_new")
        nc.tensor.matmul(ps_new, lhsT=vn_all, rhs=pd, start=True, stop=True)
        # lrow[0, 32+p] = p_new[p]; staged to SBUF so the epilogue add has
        # only one PSUM operand
        nc.tensor.matmul(lrow[0:1, npairs : 2 * npairs], lhsT=p_new, rhs=id32,
                         start=True, stop=True, skip_group_check=True)
        pnew_row = singles.tile([1, npairs], f32, tag="pnew_row")
        nc.scalar.copy(pnew_row, lrow[0:1, npairs : 2 * npairs])

        def q_bcast_ap(p):
            base = qbs[p // pairs_per_qb][:, (p % pairs_per_qb) * hd : (p % pairs_per_qb + 1) * hd]
            return bass.AP(tensor=base.tensor, offset=base.offset,
                           ap=[base.ap[0], [0, nch], base.ap[1]])

        def q_bcast_ap_n(p, n_ch):
            base = qbs[p // pairs_per_qb][:, (p % pairs_per_qb) * hd : (p % pairs_per_qb + 1) * hd]
            return bass.AP(tensor=base.tensor, offset=base.offset,
                           ap=[base.ap[0], [0, n_ch], base.ap[1]])

        def score_block(p, kt, n_ch, piece):
            """scores+exp for n_ch chunks of pair p; returns (p_tile, l_part)."""
            prod = ppool.tile([P, n_ch, hd], f16, tag="prod" if n_ch == nch else "prod31",
                              bufs=4 if n_ch == nch else 2, name=f"prod_{p}_{piece}")
            # optionally offload odd pairs' mul to gpsimd to widen DVE slack
            meng = nc.gpsimd if (MSPLIT and n_ch == nch and p % 2 == 1) else nc.vector
            meng.tensor_mul(prod, kt, q_bcast_ap_n(p, n_ch))
            s_t = stats.tile([P, n_ch], f32, tag="s" if n_ch == nch else "s31",
                             name=f"s_{p}_{piece}")
            nc.vector.reduce_sum(s_t, prod, axis=mybir.AxisListType.X)
            p_t = stats.tile([P, n_ch], f16, tag="p" if n_ch == nch else "p31",
                             name=f"p_{p}_{piece}")
            l_t = stats.tile([P, 1], f32, tag="l" if n_ch == nch else "l31",
                             name=f"l_{p}_{piece}")
            nc.scalar.activation(
                out=p_t, in_=s_t, func=mybir.ActivationFunctionType.Exp,
                scale=sm_scale, accum_out=l_t,
            )
            return p_t, l_t

        # ---- main loop: pairs 0..30 ----
        for p in range(npairs - 1):
            p_t, l_t = score_block(p, kts[p], nch, 0)

            acc = psum_acc.tile([P, 1], f32, tag="acc", name=f"acc{p}")
            for c in range(nch):
                nc.tensor.matmul(acc, lhsT=vts[p][:, c, :], rhs=p_t[:, c : c + 1],
                                 start=(c == 0), stop=(c == nch - 1))

            # denominator (cache part): lrow[0, p] = sum over partitions of l_t
            nc.tensor.matmul(lrow[0:1, p : p + 1], lhsT=ones_col, rhs=l_t,
                             start=True, stop=True, skip_group_check=True)

            nc.scalar.copy(out_sb[:, p : p + 1], acc)

        # ---- last pair, split into NSPLIT chunk-groups to shrink the tail ----
        p31 = npairs - 1
        acc = psum_acc.tile([P, 1], f32, tag="acc", name="acc31")
        for j in range(NSPLIT):
            p_t, l_t = score_block(p31, kt31[j], spc, j)
            for cc in range(spc):
                nc.tensor.matmul(acc, lhsT=vt31[j][:, cc, :], rhs=p_t[:, cc : cc + 1],
                                 start=(j == 0 and cc == 0),
                                 stop=(j == NSPLIT - 1 and cc == spc - 1),
                                 skip_group_check=True)
            nc.tensor.matmul(lrow[0:1, p31 : p31 + 1], lhsT=ones_col, rhs=l_t,
                             start=(j == 0), stop=(j == NSPLIT - 1),
                             skip_group_check=True)
        nc.scalar.copy(out_sb[:, p31 : p31 + 1], acc)

        # ---- epilogue: normalize all pairs at once, single 16 KiB store ----
        l_tot = singles.tile([1, npairs], f32, tag="l_tot")
        nc.vector.tensor_add(l_tot, lrow[0:1, 0:npairs], pnew_row)
        recip_row = singles.tile([1, npairs], f32, tag="recip_row")
        nc.vector.reciprocal(recip_row, l_tot)
        ps_rb = psum_q.tile([P, npairs], f32, tag="psq", name="ps_rb")
        nc.tensor.matmul(ps_rb, lhsT=ones_row, rhs=recip_row, start=True, stop=True)
        rb_sb = singles.tile([P, npairs], f32, tag="rb_sb")
        nc.scalar.copy(rb_sb, ps_rb)
        out_cache = singles.tile([P, npairs], f32, tag="out_cache")
        nc.vector.tensor_add(out_cache, out_sb, ps_new)
        out_final = singles.tile([P, npairs], f32, tag="out_final")
        nc.vector.tensor_mul(out_final, out_cache, rb_sb)
        nc.sync.dma_start(out=out, in_=out_final)

    nc.compile()
    return nc


@functools.lru_cache(maxsize=4)
def _program(bl, n_heads, max_seq, hd, cache_len):
    return _build_program(bl, n_heads, max_seq, hd, cache_len)


def kernel(query, key, value, key_cache, value_cache, cache_len):
    global LAST_RESULTS
    query = np.asarray(query, dtype=np.float32)
    key = np.asarray(key, dtype=np.float32)
    value = np.asarray(value, dtype=np.float32)
    key_cache = np.asarray(key_cache, dtype=np.float32)
    value_cache = np.asarray(value_cache, dtype=np.float32)
    cache_len = int(cache_len)

    b_sz, q_len, d_model = query.shape
    _, n_heads, max_seq, hd = key_cache.shape
    assert q_len == 1 and d_model == n_heads * hd
    assert b_sz % N_CORES == 0
    bl = b_sz // N_CORES

    prog = _program(bl, n_heads, max_seq, hd, cache_len)

    in_maps = []
    for i in range(N_CORES):
        sl = slice(i * bl, (i + 1) * bl)
        in_maps.append(
            {
                "kc": np.ascontiguousarray(key_cache[sl]),
                "vc": np.ascontiguousarray(value_cache[sl]),
                "q": np.ascontiguousarray(query[sl]).reshape(bl * n_heads, hd),
                "kn": np.ascontiguousarray(key[sl]).reshape(bl * n_heads, hd),
                "vn": np.ascontiguousarray(value[sl]).reshape(bl * n_heads, hd),
                "ident32": np.eye(bl * n_heads, dtype=np.float32),
            }
        )

    try:
        res = bass_utils.run_bass_kernel_spmd(
            prog, in_maps, core_ids=list(range(N_CORES)), trace=TRACE, **TRACE_KWARGS
        )
    except Exception:
        res = bass_utils.run_bass_kernel_spmd(
            prog, in_maps, core_ids=list(range(N_CORES)), trace=TRACE, **TRACE_KWARGS
        )
    LAST_RESULTS = res
    outs = [
        res.results[i]["out"].T.reshape(bl, q_len, d_model) for i in range(N_CORES)
    ]
    return np.concatenate(outs, axis=0)


# revision 3
# speedup vs baseline: 1.1184x; 1.0665x over previous
"""Decode-step multi-head attention with KV cache (DeepSpeed-inference style).

Full shapes (hardcoded per problem spec):
  query/key/value: [16, 1, 2048] f32
  key_cache/value_cache: [16, 16, 4096, 128] f32
  cache_len: 2048
Output: [16, 1, 2048] f32

Data-parallel over batch across 8 NeuronCores: 2 batches/core = 32
(batch, head) pairs per core. Per pair the core streams the K cache
slice (f32, sync HWDGE queue) and the V cache slice (cast f32->f16
during the SWDGE DMA on the gpsimd queue) from HBM; the ~370 GB/s
per-core HBM read rate is the roofline (~64 MiB/core). All 63 streaming
DMAs are emitted first so both queues drain from t=0, with deep buffer
pools so neither queue stalls on consumption. Scores are computed with
an elementwise mul (alternating DVE / GpSimd per pair, so neither
engine paces the stream) + per-chunk reduce on DVE; exp with fused
row-sum on ScalarE; V aggregation on TensorE as f16 matmuls
accumulating in PSUM. The new-token (q.k_new, v_new) contributions are
folded in with two batched diag(p_new) matmuls in the prologue.
Softmax denominators collect into a persistent [1, 64] PSUM row via
ones-matmuls; the final normalize broadcasts the reciprocal row across
partitions with a ones-matmul and multiplies on DVE. The output leaves
as [hd, npairs] and the host transposes. The last pair is split into 4
chunk-groups so the post-stream compute tail stays short.
"""

import functools
import os
from contextlib import ExitStack

import numpy as np

import concourse.bacc as bacc
import concourse.bass as bass
import concourse.mybir as mybir
import concourse.tile as tile
from concourse import bass_utils

N_CORES = 8
P = 128

TRACE = False
TRACE_KWARGS = {}
LAST_RESULTS = None


def _build_program(bl: int, n_heads: int, max_seq: int, hd: int, cache_len: int):
    npairs = bl * n_heads
    assert hd == P and cache_len % P == 0 and npairs == 32
    nch = cache_len // P  # 16
    NSPLIT = int(os.environ.get("NSPLIT", "4"))  # chunk-groups for last pair
    assert nch % NSPLIT == 0
    spc = nch // NSPLIT  # chunks per split piece
    sm_scale = 1.0 / float(np.sqrt(hd))
    KBUFS = int(os.environ.get("KBUFS", "11"))
    VBUFS = int(os.environ.get("VBUFS", "8"))
    MSPLIT = int(os.environ.get("MSPLIT", "1"))

    nc = bacc.Bacc("TRN2", target_bir_lowering=False, debug=False)
    f32 = mybir.dt.float32
    f16 = mybir.dt.float16

    kc = nc.dram_tensor("kc", [bl, n_heads, max_seq, hd], f32, kind="ExternalInput").ap()
    vc = nc.dram_tensor("vc", [bl, n_heads, max_seq, hd], f32, kind="ExternalInput").ap()
    q = nc.dram_tensor("q", [npairs, hd], f32, kind="ExternalInput").ap()
    kn = nc.dram_tensor("kn", [npairs, hd], f32, kind="ExternalInput").ap()
    vn = nc.dram_tensor("vn", [npairs, hd], f32, kind="ExternalInput").ap()
    ident32 = nc.dram_tensor("ident32", [npairs, npairs], f32, kind="ExternalInput").ap()
    out = nc.dram_tensor("out", [hd, npairs], f32, kind="ExternalOutput").ap()

    with tile.TileContext(nc) as tc, ExitStack() as ctx:
        singles = ctx.enter_context(tc.tile_pool(name="singles", bufs=1))
        kpool = ctx.enter_context(tc.tile_pool(name="kpool", bufs=KBUFS))
        vpool = ctx.enter_context(tc.tile_pool(name="vpool", bufs=VBUFS))
        ppool = ctx.enter_context(tc.tile_pool(name="ppool", bufs=4))
        stats = ctx.enter_context(tc.tile_pool(name="stats", bufs=10))
        psum_acc = ctx.enter_context(tc.tile_pool(name="psum_acc", bufs=3, space="PSUM"))
        psum_new = ctx.enter_context(tc.tile_pool(name="psum_new", bufs=1, space="PSUM"))
        psum_q = ctx.enter_context(tc.tile_pool(name="psum_q", bufs=2, space="PSUM"))
        psum_l = ctx.enter_context(tc.tile_pool(name="psum_l", bufs=1, space="PSUM"))

        # ---- streaming loads: K f32 on sync HWDGE, V f32->f16 cast on
        # gpsimd SWDGE (16-bit V keeps the PE matmul weight loads fast).
        # Emitted first so both queues start draining at t=0; buffer reuse
        # (KBUFS/VBUFS) self-regulates queue depth. ----
        def emit_k(b, h, lo_ch, n_ch, tag, bufs):
            t = kpool.tile([P, n_ch, hd], f32, tag=tag, bufs=bufs, name=f"kt_{b}_{h}_{lo_ch}")
            src = kc[b, h, 0 : nch * P, :].rearrange("(p c) d -> p c d", c=nch)
            nc.sync.dma_start(out=t, in_=src[:, lo_ch : lo_ch + n_ch, :])
            return t

        def emit_v(b, h, lo_ch, n_ch, tag, bufs):
            t = vpool.tile([P, n_ch, hd], f16, tag=tag, bufs=bufs, name=f"vt_{b}_{h}_{lo_ch}")
            src = vc[b, h, 0 : nch * P, :].rearrange("(p c) d -> p c d", c=nch)
            nc.gpsimd.dma_start(out=t, in_=src[:, lo_ch : lo_ch + n_ch, :])
            return t

        kts, vts = [], []
        for p in range(npairs - 1):
            b, h = divmod(p, n_heads)
            kts.append(emit_k(b, h, 0, nch, "kt", KBUFS))
            vts.append(emit_v(b, h, 0, nch, "vt", VBUFS))
        b31, h31 = divmod(npairs - 1, n_heads)
        kt31 = [emit_k(b31, h31, j * spc, spc, "kt31", NSPLIT) for j in range(NSPLIT)]
        vt31 = [emit_v(b31, h31, j * spc, spc, "vt31", NSPLIT) for j in range(NSPLIT)]

        # ---- setup: small loads on the scalar HWDGE queue ----
        q_flat = singles.tile([1, npairs * hd], f32, tag="q_flat")
        q_flat_src = bass.AP(tensor=q.tensor, offset=q.offset, ap=[[0, 1], [1, npairs * hd]])
        nc.scalar.dma_start(out=q_flat, in_=q_flat_src)
        q_all = singles.tile([npairs, hd], f32, tag="q_all")
        nc.scalar.dma_start(out=q_all, in_=q)
        kn_all = singles.tile([npairs, hd], f32, tag="kn_all")
        nc.scalar.dma_start(out=kn_all, in_=kn)
        vn_all = singles.tile([npairs, hd], f32, tag="vn_all")
        nc.scalar.dma_start(out=vn_all, in_=vn)
        id32 = singles.tile([npairs, npairs], f32, tag="id32")
        nc.scalar.dma_start(out=id32, in_=ident32)

        ones_row = singles.tile([1, P], f32, tag="ones_row")
        nc.vector.memset(ones_row, 1.0)
        ones_col = singles.tile([P, 1], f32, tag="ones_col")
        nc.vector.memset(ones_col, 1.0)

        # ---- q broadcast to all partitions via PE ones-matmul:
        # qb[j][k, d'] = q_flat[0, j*512 + d'] for every partition k ----
        QCOLS = 512
        nqb = (npairs * hd) // QCOLS  # 8 blocks of 4 pairs each
        pairs_per_qb = QCOLS // hd  # 4
        qbs = []
        for j in range(nqb):
            psq = psum_q.tile([P, QCOLS], f32, tag="psq", name=f"psq{j}")
            nc.tensor.matmul(
                psq, lhsT=ones_row, rhs=q_flat[0:1, j * QCOLS : (j + 1) * QCOLS],
                start=True, stop=True,
            )
            qb = singles.tile([P, QCOLS], f32, tag=f"qb{j}", name=f"qb{j}")
            nc.scalar.copy(qb, psq)
            qbs.append(qb)

        # ---- new-token scores: p_new[p] = exp(q[p]·kn[p] * scale) ----
        prod_new = singles.tile([npairs, hd], f32, tag="prod_new")
        nc.vector.tensor_mul(prod_new, kn_all, q_all)
        s_new = singles.tile([npairs, 1], f32, tag="s_new")
        nc.vector.reduce_sum(s_new, prod_new, axis=mybir.AxisListType.X)
        p_new = singles.tile([npairs, 1], f32, tag="p_new")
        nc.scalar.activation(
            out=p_new, in_=s_new, func=mybir.ActivationFunctionType.Exp, scale=sm_scale
        )

        # Softmax denominators, partition 0: cols 0..31 = cache part (one
        # per pair), cols 32..63 = transposed p_new row. Persistent.
        lrow = psum_l.tile([1, 2 * npairs], f32, tag="lrow")
        # Unnormalized outputs, head-dim on partitions, one column per pair.
        out_sb = singles.tile([P, npairs], f32, tag="out_sb")

        # ---- new-token contributions, batched via diag(p_new) matmuls ----
        pd = singles.tile([npairs, npairs], f32, tag="pd")
        nc.vector.tensor_scalar_mul(pd, id32, p_new)
        # ps_new[d, p] = p_new[p] * vn[p, d]
        ps_new = psum_new.tile([P, npairs], f32, tag="ps_new")
        nc.tensor.matmul(ps_new, lhsT=vn_all, rhs=pd, start=True, stop=True)
        # lrow[0, 32+p] = p_new[p]; staged to SBUF so the epilogue add has
        # only one PSUM operand
        nc.tensor.matmul(lrow[0:1, npairs : 2 * npairs], lhsT=p_new, rhs=id32,
                         start=True, stop=True, skip_group_check=True)
        pnew_row = singles.tile([1, npairs], f32, tag="pnew_row")
        nc.scalar.copy(pnew_row, lrow[0:1, npairs : 2 * npairs])

        def q_bcast_ap(p):
            base = qbs[p // pairs_per_qb][:, (p % pairs_per_qb) * hd : (p % pairs_per_qb + 1) * hd]
            return bass.AP(tensor=base.tensor, offset=base.offset,
                           ap=[base.ap[0], [0, nch], base.ap[1]])

        def q_bcast_ap_n(p, n_ch):
            base = qbs[p // pairs_per_qb][:, (p % pairs_per_qb) * hd : (p % pairs_per_qb + 1) * hd]
            return bass.AP(tensor=base.tensor, offset=base.offset,
                           ap=[base.ap[0], [0, n_ch], base.ap[1]])

        def score_block(p, kt, n_ch, piece):
            """scores+exp for n_ch chunks of pair p; returns (p_tile, l_part)."""
            prod = ppool.tile([P, n_ch, hd], f16, tag="prod" if n_ch == nch else "prod31",
                              bufs=4 if n_ch == nch else 2, name=f"prod_{p}_{piece}")
            # optionally offload odd pairs' mul to gpsimd to widen DVE slack
            meng = nc.gpsimd if (MSPLIT and n_ch == nch and p % 2 == 1) else nc.vector
            meng.tensor_mul(prod, kt, q_bcast_ap_n(p, n_ch))
            s_t = stats.tile([P, n_ch], f32, tag="s" if n_ch == nch else "s31",
                             name=f"s_{p}_{piece}")
            nc.vector.reduce_sum(s_t, prod, axis=mybir.AxisListType.X)
            p_t = stats.tile([P, n_ch], f16, tag="p" if n_ch == nch else "p31",
                             name=f"p_{p}_{piece}")
            l_t = stats.tile([P, 1], f32, tag="l" if n_ch == nch else "l31",
                             name=f"l_{p}_{piece}")
            nc.scalar.activation(
                out=p_t, in_=s_t, func=mybir.ActivationFunctionType.Exp,
                scale=sm_scale, accum_out=l_t,
            )
            return p_t, l_t

        # ---- main loop: pairs 0..30 ----
        for p in range(npairs - 1):
            p_t, l_t = score_block(p, kts[p], nch, 0)

            acc = psum_acc.tile([P, 1], f32, tag="acc", name=f"acc{p}")
            for c in range(nch):
                nc.tensor.matmul(acc, lhsT=vts[p][:, c, :], rhs=p_t[:, c : c + 1],
                                 start=(c == 0), stop=(c == nch - 1))

            # denominator (cache part): lrow[0, p] = sum over partitions of l_t
            nc.tensor.matmul(lrow[0:1, p : p + 1], lhsT=ones_col, rhs=l_t,
                             start=True, stop=True, skip_group_check=True)

            nc.scalar.copy(out_sb[:, p : p + 1], acc)

        # ---- last pair, split into NSPLIT chunk-groups to shrink the tail ----
        p31 = npairs - 1
        acc = psum_acc.tile([P, 1], f32, tag="acc", name="acc31")
        for j in range(NSPLIT):
            p_t, l_t = score_block(p31, kt31[j], spc, j)
            for cc in range(spc):
                nc.tensor.matmul(acc, lhsT=vt31[j][:, cc, :], rhs=p_t[:, cc : cc + 1],
                                 start=(j == 0 and cc == 0),
                                 stop=(j == NSPLIT - 1 and cc == spc - 1),
                                 skip_group_check=True)
            nc.tensor.matmul(lrow[0:1, p31 : p31 + 1], lhsT=ones_col, rhs=l_t,
                             start=(j == 0), stop=(j == NSPLIT - 1),
                             skip_group_check=True)
        nc.scalar.copy(out_sb[:, p31 : p31 + 1], acc)

        # ---- epilogue: normalize all pairs at once, single 16 KiB store ----
        l_tot = singles.tile([1, npairs], f32, tag="l_tot")
        nc.vector.tensor_add(l_tot, lrow[0:1, 0:npairs], pnew_row)
        recip_row = singles.tile([1, npairs], f32, tag="recip_row")
        nc.vector.reciprocal(recip_row, l_tot)
        ps_rb = psum_q.tile([P, npairs], f32, tag="psq", name="ps_rb")
        nc.tensor.matmul(ps_rb, lhsT=ones_row, rhs=recip_row, start=True, stop=True)
        rb_sb = singles.tile([P, npairs], f32, tag="rb_sb")
        nc.scalar.copy(rb_sb, ps_rb)
        out_cache = singles.tile([P, npairs], f32, tag="out_cache")
        nc.vector.tensor_add(out_cache, out_sb, ps_new)
        out_final = singles.tile([P, npairs], f32, tag="out_final")
        nc.vector.tensor_mul(out_final, out_cache, rb_sb)
        nc.sync.dma_start(out=out, in_=out_final)

    nc.compile()
    return nc


@functools.lru_cache(maxsize=4)
def _program(bl, n_heads, max_seq, hd, cache_len):
    return _build_program(bl, n_heads, max_seq, hd, cache_len)


def kernel(query, key, value, key_cache, value_cache, cache_len):
    global LAST_RESULTS
    query = np.asarray(query, dtype=np.float32)
    key = np.asarray(key, dtype=np.float32)
    value = np.asarray(value, dtype=np.float32)
    key_cache = np.asarray(key_cache, dtype=np.float32)
    value_cache = np.asarray(value_cache, dtype=np.float32)
    cache_len = int(cache_len)

    b_sz, q_len, d_model = query.shape
    _, n_heads, max_seq, hd = key_cache.shape
    assert q_len == 1 and d_model == n_heads * hd
    assert b_sz % N_CORES == 0
    bl = b_sz // N_CORES

    prog = _program(bl, n_heads, max_seq, hd, cache_len)

    in_maps = []
    for i in range(N_CORES):
        sl = slice(i * bl, (i + 1) * bl)
        in_maps.append(
            {
                "kc": np.ascontiguousarray(key_cache[sl]),
                "vc": np.ascontiguousarray(value_cache[sl]),
                "q": np.ascontiguousarray(query[sl]).reshape(bl * n_heads, hd),
                "kn": np.ascontiguousarray(key[sl]).reshape(bl * n_heads, hd),
                "vn": np.ascontiguousarray(value[sl]).reshape(bl * n_heads, hd),
                "ident32": np.eye(bl * n_heads, dtype=np.float32),
            }
        )

    try:
        res = bass_utils.run_bass_kernel_spmd(
            prog, in_maps, core_ids=list(range(N_CORES)), trace=TRACE, **TRACE_KWARGS
        )
    except Exception:
        res = bass_utils.run_bass_kernel_spmd(
            prog, in_maps, core_ids=list(range(N_CORES)), trace=TRACE, **TRACE_KWARGS
        )
    LAST_RESULTS = res
    outs = [
        res.results[i]["out"].T.reshape(bl, q_len, d_model) for i in range(N_CORES)
    ]
    return np.concatenate(outs, axis=0)


# revision 4
# speedup vs baseline: 1.1684x; 1.0447x over previous
"""Decode-step multi-head attention with KV cache (DeepSpeed-inference style).

Full shapes (hardcoded per problem spec):
  query/key/value: [16, 1, 2048] f32
  key_cache/value_cache: [16, 16, 4096, 128] f32
  cache_len: 2048
Output: [16, 1, 2048] f32

Data-parallel over batch across 8 NeuronCores: 2 batches/core = 32
(batch, head) pairs per core. Per pair the core streams the K cache
slice (f32, sync HWDGE queue) and the V cache slice (cast f32->f16
during the SWDGE DMA on the gpsimd queue) from HBM; the ~370 GB/s
per-core HBM read rate is the roofline (~64 MiB/core). K issues are
emitted upfront; V issues are interleaved into the pair loop so a
V-issue waiting on buffer recycling never blocks the gpsimd engine's
score muls behind it (FIFO head-of-line). Scores are computed with an
elementwise mul (alternating DVE / GpSimd per pair, so neither engine
paces the stream) + per-chunk reduce on DVE; exp with fused row-sum on
ScalarE; V aggregation on TensorE as f16 matmuls accumulating in PSUM.
The new-token (q.k_new, v_new) contributions are folded in with two
batched diag(p_new) matmuls in the prologue. Softmax denominators
collect into a persistent [1, 64] PSUM row via ones-matmuls; the final
normalize broadcasts the reciprocal row across partitions with a
ones-matmul and multiplies on DVE. The output leaves as [hd, npairs]
and the host transposes. The last pair is split into 4 chunk-groups so
the post-stream compute tail stays short.
"""

import functools
import os
from contextlib import ExitStack

import numpy as np

import concourse.bacc as bacc
import concourse.bass as bass
import concourse.mybir as mybir
import concourse.tile as tile
from concourse import bass_utils

N_CORES = 8
P = 128

TRACE = False
TRACE_KWARGS = {}
LAST_RESULTS = None


def _build_program(bl: int, n_heads: int, max_seq: int, hd: int, cache_len: int):
    npairs = bl * n_heads
    assert hd == P and cache_len % P == 0 and npairs == 32
    nch = cache_len // P  # 16
    NSPLIT = int(os.environ.get("NSPLIT", "4"))  # chunk-groups for last pair
    assert nch % NSPLIT == 0
    spc = nch // NSPLIT  # chunks per split piece
    sm_scale = 1.0 / float(np.sqrt(hd))
    KBUFS = int(os.environ.get("KBUFS", "11"))
    VBUFS = int(os.environ.get("VBUFS", "10"))
    MSPLIT = int(os.environ.get("MSPLIT", "1"))

    nc = bacc.Bacc("TRN2", target_bir_lowering=False, debug=False)
    f32 = mybir.dt.float32
    f16 = mybir.dt.float16

    kc = nc.dram_tensor("kc", [bl, n_heads, max_seq, hd], f32, kind="ExternalInput").ap()
    vc = nc.dram_tensor("vc", [bl, n_heads, max_seq, hd], f32, kind="ExternalInput").ap()
    q = nc.dram_tensor("q", [npairs, hd], f32, kind="ExternalInput").ap()
    kn = nc.dram_tensor("kn", [npairs, hd], f32, kind="ExternalInput").ap()
    vn = nc.dram_tensor("vn", [npairs, hd], f32, kind="ExternalInput").ap()
    ident32 = nc.dram_tensor("ident32", [npairs, npairs], f32, kind="ExternalInput").ap()
    out = nc.dram_tensor("out", [hd, npairs], f32, kind="ExternalOutput").ap()

    with tile.TileContext(nc) as tc, ExitStack() as ctx:
        singles = ctx.enter_context(tc.tile_pool(name="singles", bufs=1))
        kpool = ctx.enter_context(tc.tile_pool(name="kpool", bufs=KBUFS))
        vpool = ctx.enter_context(tc.tile_pool(name="vpool", bufs=VBUFS))
        ppool = ctx.enter_context(tc.tile_pool(name="ppool", bufs=4))
        stats = ctx.enter_context(tc.tile_pool(name="stats", bufs=10))
        psum_acc = ctx.enter_context(tc.tile_pool(name="psum_acc", bufs=3, space="PSUM"))
        psum_new = ctx.enter_context(tc.tile_pool(name="psum_new", bufs=1, space="PSUM"))
        psum_q = ctx.enter_context(tc.tile_pool(name="psum_q", bufs=2, space="PSUM"))
        psum_l = ctx.enter_context(tc.tile_pool(name="psum_l", bufs=1, space="PSUM"))

        # ---- streaming loads: K f32 on sync HWDGE, V f32->f16 cast on
        # gpsimd SWDGE (16-bit V keeps the PE matmul weight loads fast).
        # Emitted first so both queues start draining at t=0; buffer reuse
        # (KBUFS/VBUFS) self-regulates queue depth. ----
        def emit_k(b, h, lo_ch, n_ch, tag, bufs):
            t = kpool.tile([P, n_ch, hd], f32, tag=tag, bufs=bufs, name=f"kt_{b}_{h}_{lo_ch}")
            src = kc[b, h, 0 : nch * P, :].rearrange("(p c) d -> p c d", c=nch)
            nc.sync.dma_start(out=t, in_=src[:, lo_ch : lo_ch + n_ch, :])
            return t

        def emit_v(b, h, lo_ch, n_ch, tag, bufs):
            t = vpool.tile([P, n_ch, hd], f16, tag=tag, bufs=bufs, name=f"vt_{b}_{h}_{lo_ch}")
            src = vc[b, h, 0 : nch * P, :].rearrange("(p c) d -> p c d", c=nch)
            nc.gpsimd.dma_start(out=t, in_=src[:, lo_ch : lo_ch + n_ch, :])
            return t

        # K issues all upfront (the sync engine has no compute queued behind
        # them). V issues only up to the buffer depth here: the rest are
        # emitted inside the pair loop, so a V-issue waiting on buffer
        # recycling never holds the gpsimd engine's score muls hostage
        # (FIFO head-of-line).
        kts, vts = [], []
        for p in range(npairs - 1):
            b, h = divmod(p, n_heads)
            kts.append(emit_k(b, h, 0, nch, "kt", KBUFS))
        b31, h31 = divmod(npairs - 1, n_heads)
        kt31 = [emit_k(b31, h31, j * spc, spc, "kt31", NSPLIT) for j in range(NSPLIT)]
        for p in range(min(VBUFS, npairs - 1)):
            b, h = divmod(p, n_heads)
            vts.append(emit_v(b, h, 0, nch, "vt", VBUFS))
        vt31 = []

        def top_up_v(p):
            nv = p + VBUFS
            if nv <= npairs - 2:
                b, h = divmod(nv, n_heads)
                vts.append(emit_v(b, h, 0, nch, "vt", VBUFS))
            elif nv - (npairs - 1) < NSPLIT:
                j = nv - (npairs - 1)
                vt31.append(emit_v(b31, h31, j * spc, spc, "vt31", NSPLIT))

        # ---- setup: small loads on the scalar HWDGE queue ----
        q_flat = singles.tile([1, npairs * hd], f32, tag="q_flat")
        q_flat_src = bass.AP(tensor=q.tensor, offset=q.offset, ap=[[0, 1], [1, npairs * hd]])
        nc.scalar.dma_start(out=q_flat, in_=q_flat_src)
        q_all = singles.tile([npairs, hd], f32, tag="q_all")
        nc.scalar.dma_start(out=q_all, in_=q)
        kn_all = singles.tile([npairs, hd], f32, tag="kn_all")
        nc.scalar.dma_start(out=kn_all, in_=kn)
        vn_all = singles.tile([npairs, hd], f32, tag="vn_all")
        nc.scalar.dma_start(out=vn_all, in_=vn)
        id32 = singles.tile([npairs, npairs], f32, tag="id32")
        nc.scalar.dma_start(out=id32, in_=ident32)

        ones_row = singles.tile([1, P], f32, tag="ones_row")
        nc.vector.memset(ones_row, 1.0)
        ones_col = singles.tile([P, 1], f32, tag="ones_col")
        nc.vector.memset(ones_col, 1.0)

        # ---- q broadcast to all partitions via PE ones-matmul:
        # qb[j][k, d'] = q_flat[0, j*512 + d'] for every partition k ----
        QCOLS = 512
        nqb = (npairs * hd) // QCOLS  # 8 blocks of 4 pairs each
        pairs_per_qb = QCOLS // hd  # 4
        qbs = []
        for j in range(nqb):
            psq = psum_q.tile([P, QCOLS], f32, tag="psq", name=f"psq{j}")
            nc.tensor.matmul(
                psq, lhsT=ones_row, rhs=q_flat[0:1, j * QCOLS : (j + 1) * QCOLS],
                start=True, stop=True,
            )
            qb = singles.tile([P, QCOLS], f32, tag=f"qb{j}", name=f"qb{j}")
            nc.scalar.copy(qb, psq)
            qbs.append(qb)

        # ---- new-token scores: p_new[p] = exp(q[p]·kn[p] * scale) ----
        prod_new = singles.tile([npairs, hd], f32, tag="prod_new")
        nc.vector.tensor_mul(prod_new, kn_all, q_all)
        s_new = singles.tile([npairs, 1], f32, tag="s_new")
        nc.vector.reduce_sum(s_new, prod_new, axis=mybir.AxisListType.X)
        p_new = singles.tile([npairs, 1], f32, tag="p_new")
        nc.scalar.activation(
            out=p_new, in_=s_new, func=mybir.ActivationFunctionType.Exp, scale=sm_scale
        )

        # Softmax denominators, partition 0: cols 0..31 = cache part (one
        # per pair), cols 32..63 = transposed p_new row. Persistent.
        lrow = psum_l.tile([1, 2 * npairs], f32, tag="lrow")
        # Unnormalized outputs, head-dim on partitions, one column per pair.
        out_sb = singles.tile([P, npairs], f32, tag="out_sb")

        # ---- new-token contributions, batched via diag(p_new) matmuls ----
        pd = singles.tile([npairs, npairs], f32, tag="pd")
        nc.vector.tensor_scalar_mul(pd, id32, p_new)
        # ps_new[d, p] = p_new[p] * vn[p, d]
        ps_new = psum_new.tile([P, npairs], f32, tag="ps_new")
        nc.tensor.matmul(ps_new, lhsT=vn_all, rhs=pd, start=True, stop=True)
        # lrow[0, 32+p] = p_new[p]; staged to SBUF so the epilogue add has
        # only one PSUM operand
        nc.tensor.matmul(lrow[0:1, npairs : 2 * npairs], lhsT=p_new, rhs=id32,
                         start=True, stop=True, skip_group_check=True)
        pnew_row = singles.tile([1, npairs], f32, tag="pnew_row")
        nc.scalar.copy(pnew_row, lrow[0:1, npairs : 2 * npairs])

        def q_bcast_ap(p):
            base = qbs[p // pairs_per_qb][:, (p % pairs_per_qb) * hd : (p % pairs_per_qb + 1) * hd]
            return bass.AP(tensor=base.tensor, offset=base.offset,
                           ap=[base.ap[0], [0, nch], base.ap[1]])

        def q_bcast_ap_n(p, n_ch):
            base = qbs[p // pairs_per_qb][:, (p % pairs_per_qb) * hd : (p % pairs_per_qb + 1) * hd]
            return bass.AP(tensor=base.tensor, offset=base.offset,
                           ap=[base.ap[0], [0, n_ch], base.ap[1]])

        def score_block(p, kt, n_ch, piece):
            """scores+exp for n_ch chunks of pair p; returns (p_tile, l_part)."""
            prod = ppool.tile([P, n_ch, hd], f16, tag="prod" if n_ch == nch else "prod31",
                              bufs=4 if n_ch == nch else 2, name=f"prod_{p}_{piece}")
            # optionally offload odd pairs' mul to gpsimd to widen DVE slack
            meng = nc.gpsimd if (MSPLIT and n_ch == nch and p % 2 == 1) else nc.vector
            meng.tensor_mul(prod, kt, q_bcast_ap_n(p, n_ch))
            s_t = stats.tile([P, n_ch], f32, tag="s" if n_ch == nch else "s31",
                             name=f"s_{p}_{piece}")
            nc.vector.reduce_sum(s_t, prod, axis=mybir.AxisListType.X)
            p_t = stats.tile([P, n_ch], f16, tag="p" if n_ch == nch else "p31",
                             name=f"p_{p}_{piece}")
            l_t = stats.tile([P, 1], f32, tag="l" if n_ch == nch else "l31",
                             name=f"l_{p}_{piece}")
            nc.scalar.activation(
                out=p_t, in_=s_t, func=mybir.ActivationFunctionType.Exp,
                scale=sm_scale, accum_out=l_t,
            )
            return p_t, l_t

        # ---- main loop: pairs 0..30 ----
        for p in range(npairs - 1):
            p_t, l_t = score_block(p, kts[p], nch, 0)

            acc = psum_acc.tile([P, 1], f32, tag="acc", name=f"acc{p}")
            for c in range(nch):
                nc.tensor.matmul(acc, lhsT=vts[p][:, c, :], rhs=p_t[:, c : c + 1],
                                 start=(c == 0), stop=(c == nch - 1))

            # denominator (cache part): lrow[0, p] = sum over partitions of l_t
            nc.tensor.matmul(lrow[0:1, p : p + 1], lhsT=ones_col, rhs=l_t,
                             start=True, stop=True, skip_group_check=True)

            nc.scalar.copy(out_sb[:, p : p + 1], acc)
            top_up_v(p)

        # ---- last pair, split into NSPLIT chunk-groups to shrink the tail ----
        for p in range(npairs - 1, npairs - 1 + NSPLIT):
            top_up_v(p)  # emit any remaining V pieces
        p31 = npairs - 1
        acc = psum_acc.tile([P, 1], f32, tag="acc", name="acc31")
        for j in range(NSPLIT):
            p_t, l_t = score_block(p31, kt31[j], spc, j)
            for cc in range(spc):
                nc.tensor.matmul(acc, lhsT=vt31[j][:, cc, :], rhs=p_t[:, cc : cc + 1],
                                 start=(j == 0 and cc == 0),
                                 stop=(j == NSPLIT - 1 and cc == spc - 1),
                                 skip_group_check=True)
            nc.tensor.matmul(lrow[0:1, p31 : p31 + 1], lhsT=ones_col, rhs=l_t,
                             start=(j == 0), stop=(j == NSPLIT - 1),
                             skip_group_check=True)
        nc.scalar.copy(out_sb[:, p31 : p31 + 1], acc)

        # ---- epilogue: normalize all pairs at once, single 16 KiB store ----
        l_tot = singles.tile([1, npairs], f32, tag="l_tot")
        nc.vector.tensor_add(l_tot, lrow[0:1, 0:npairs], pnew_row)
        recip_row = singles.tile([1, npairs], f32, tag="recip_row")
        nc.vector.reciprocal(recip_row, l_tot)
        ps_rb = psum_q.tile([P, npairs], f32, tag="psq", name="ps_rb")
        nc.tensor.matmul(ps_rb, lhsT=ones_row, rhs=recip_row, start=True, stop=True)
        rb_sb = singles.tile([P, npairs], f32, tag="rb_sb")
        nc.scalar.copy(rb_sb, ps_rb)
        out_cache = singles.tile([P, npairs], f32, tag="out_cache")
        nc.vector.tensor_add(out_cache, out_sb, ps_new)
        out_final = singles.tile([P, npairs], f32, tag="out_final")
        nc.vector.tensor_mul(out_final, out_cache, rb_sb)
        nc.sync.dma_start(out=out, in_=out_final)

    nc.compile()
    return nc


@functools.lru_cache(maxsize=4)
def _program(bl, n_heads, max_seq, hd, cache_len):
    return _build_program(bl, n_heads, max_seq, hd, cache_len)


def kernel(query, key, value, key_cache, value_cache, cache_len):
    global LAST_RESULTS
    query = np.asarray(query, dtype=np.float32)
    key = np.asarray(key, dtype=np.float32)
    value = np.asarray(value, dtype=np.float32)
    key_cache = np.asarray(key_cache, dtype=np.float32)
    value_cache = np.asarray(value_cache, dtype=np.float32)
    cache_len = int(cache_len)

    b_sz, q_len, d_model = query.shape
    _, n_heads, max_seq, hd = key_cache.shape
    assert q_len == 1 and d_model == n_heads * hd
    assert b_sz % N_CORES == 0
    bl = b_sz // N_CORES

    prog = _program(bl, n_heads, max_seq, hd, cache_len)

    in_maps = []
    for i in range(N_CORES):
        sl = slice(i * bl, (i + 1) * bl)
        in_maps.append(
            {
                "kc": np.ascontiguousarray(key_cache[sl]),
                "vc": np.ascontiguousarray(value_cache[sl]),
                "q": np.ascontiguousarray(query[sl]).reshape(bl * n_heads, hd),
                "kn": np.ascontiguousarray(key[sl]).reshape(bl * n_heads, hd),
                "vn": np.ascontiguousarray(value[sl]).reshape(bl * n_heads, hd),
                "ident32": np.eye(bl * n_heads, dtype=np.float32),
            }
        )

    try:
        res = bass_utils.run_bass_kernel_spmd(
            prog, in_maps, core_ids=list(range(N_CORES)), trace=TRACE, **TRACE_KWARGS
        )
    except Exception:
        res = bass_utils.run_bass_kernel_spmd(
            prog, in_maps, core_ids=list(range(N_CORES)), trace=TRACE, **TRACE_KWARGS
        )
    LAST_RESULTS = res
    outs = [
        res.results[i]["out"].T.reshape(bl, q_len, d_model) for i in range(N_CORES)
    ]
    return np.concatenate(outs, axis=0)
